# revision 12
# baseline (speedup 1.0000x reference)
"""GCN NodeAttributeAggregator on 8 Trainium2 NeuronCores.

Strategy (node-sharded, dst-partitioned edges):
  - Host precomputes index metadata: degrees (with self-loops), dinv=rsqrt(deg),
    per-core edge lists partitioned by dst owner, grouped by (dst-block of 128,
    src<32768 vs src>=32768 for int16 gather indices), padded to 128-edge tiles.
  - Device per core: dma_gather rows of a pre-scaled node table (xs = x*dinv),
    scatter-by-matmul: one-hot P matrices (built on DVE via iota + is_equal)
    contract each 128-edge tile into a 128-dst PSUM accumulator.
  - Gathers are issued as flat GCH-tile chunks per table plane (A = rows
    [0, split), B = last nhi rows), spanning dst-block boundaries, so SWDGE
    descriptor-generation on the Pool engine amortizes its fixed cost and the
    DMA engines stay ahead of the PE.
  - Dense 256x256 matmuls run in float32r (full PE rate) feature-major, with
    PE transposes at layout boundaries; the dense pass is interleaved every
    4 dst-blocks so stores overlap the aggregation stream.
  - Algebra: GCN layer out = D^-1/2 (A+I) D^-1/2 h W.  Row scaling commutes
    with right matmuls, relu commutes with positive row scaling, and
    agg(h W) = agg(h) W, so:
      L1 (fused W_pre@W1): u' = (scatter(xs) + xs_dst) * dinv;
          g1 = relu(u' @ (W_pre W1) + b1 + rank1(b_pre)) * dinv
      L2: v' = (scatter(g1) + g1_dst) * dinv;
          y  = relu(v' @ W2 + b2) @ W_post + b_post
  - Two SPMD launches; host concatenates g1 slices between them.  Both
    launches write bf16 outputs (g1 is re-quantized to bf16 tables anyway).
"""

import dataclasses
import ml_dtypes
import numpy as np

import concourse.bacc as bacc
import concourse.bass as bass
import concourse.tile as tile
import concourse.mybir as mybir
from concourse.bass_utils import run_bass_kernel_spmd
from concourse.masks import make_identity

P = 128
NSWQ = 4  # SWDGE queues (ucode max 4)
GCH = 8  # gather tiles per dma_gather call (HW ring: 1024 descs/queue)
LOOK = 2  # blocks of gather lookahead
DENSE_INTERLEAVE = True
DEBUG_U = False
f32 = mybir.dt.float32
f32r = mybir.dt.float32r
bf16 = mybir.dt.bfloat16
gdt = bf16
i16 = mybir.dt.int16
i32 = mybir.dt.int32


@dataclasses.dataclass
class Cfg:
    n_nodes: int = 50000
    d: int = 256
    nc: int = 8
    split: int = 32768
    dense_n: int = 512

    @property
    def nloc(self):
        return self.n_nodes // self.nc

    @property
    def nblk(self):
        return (self.nloc + P - 1) // P

    @property
    def npad(self):
        return self.nblk * P

    @property
    def nhi(self):
        # table B spans the last min(32768, n) rows (full int16 window)
        return min(self.split, self.n_nodes)


# ---------------------------------------------------------------- host prep


def _wrap16(vals):
    """[n] -> [128, n//16] int16: value i at [i%16, i//16], replicated x8."""
    w = vals.reshape(-1, 16).T
    return np.tile(w, (8, 1)).astype(np.int16)


def _prep_edges(cfg, src, dst):
    """Partition edges by dst owner; split per dst-block into two gather
    groups (table A = rows [0, split); table B = rows [baseB, n)).  The cut
    is chosen per block at an exact multiple of 128 edges in src-order when
    the overlap window [baseB, split) allows, so group A has no padding.

    Emits per-core flat gather planes (idxA: all A tiles block-major,
    idxB likewise) plus a merged slot plane (block-major, A tiles then B
    tiles within a block) with identical compile-time schedule (TA, TB)
    across cores.
    """
    nl, nb = cfg.nloc, cfg.nblk
    baseB = cfg.n_nodes - cfg.nhi
    owner = dst // nl
    loc = dst - owner * nl
    blk = loc // P
    slot = loc - blk * P

    key = owner * nb + blk
    nkeys = cfg.nc * nb
    n_cb = np.bincount(key, minlength=nkeys).reshape(cfg.nc, nb)
    cntlo = np.bincount(key[src < baseB], minlength=nkeys).reshape(cfg.nc, nb)
    cntA = np.bincount(key[src < cfg.split], minlength=nkeys).reshape(cfg.nc, nb)

    lo = ((cntlo + P - 1) // P).max(axis=0)  # [nb] min feasible TA
    hi = (cntA // P).min(axis=0)             # [nb] max feasible TA
    feasible = lo <= hi
    # prefer max A (A has no pad); fallback: threshold split at `split`
    TA = np.where(feasible, hi, ((cntA + P - 1) // P).max(axis=0))
    # B count per (c,b): n - A_count; A_count = TA*P if feasible else cntA
    A_cnt = np.where(feasible[None, :], np.minimum(TA[None, :] * P, n_cb),
                     cntA)
    B_cnt = n_cb - A_cnt
    TB = ((B_cnt + P - 1) // P).max(axis=0)

    cumTA = np.concatenate([[0], np.cumsum(TA)])
    cumTB = np.concatenate([[0], np.cumsum(TB)])
    nAt, nBt = int(cumTA[-1]), int(cumTB[-1])

    # rank edges within (core, block) by src (stable) to apply the cut
    order = np.lexsort((src, key))
    skey = key[order]
    group_start = np.concatenate(
        [[0], np.cumsum(np.bincount(skey, minlength=nkeys))])
    rank = np.arange(len(src)) - group_start[skey]

    ocore = owner[order]
    oblk = blk[order]
    acut = A_cnt[ocore, oblk]
    in_a = rank < acut
    rowpos = np.where(in_a, rank, rank - acut)
    rowsA = cumTA[oblk] * P + rowpos
    rowsB = cumTB[oblk] * P + rowpos
    idxv = (src[order] - np.where(in_a, 0, baseB)).astype(np.int16)
    slotv = slot[order].astype(np.float32)
    assert (src[order][in_a] < cfg.split).all()
    assert (src[order][~in_a] >= baseB).all()

    idxA = np.zeros((cfg.nc, nAt * P), np.int16)
    idxB = np.zeros((cfg.nc, nBt * P), np.int16)
    slotA = np.full((cfg.nc, nAt * P), 300.0, np.float32)
    slotB = np.full((cfg.nc, nBt * P), 300.0, np.float32)
    idxA[ocore[in_a], rowsA[in_a]] = idxv[in_a]
    idxB[ocore[~in_a], rowsB[~in_a]] = idxv[~in_a]
    slotA[ocore[in_a], rowsA[in_a]] = slotv[in_a]
    slotB[ocore[~in_a], rowsB[~in_a]] = slotv[~in_a]

    # merged slot plane: block-major, A tiles then B tiles within a block
    ntiles = nAt + nBt
    per_core = []
    for c in range(cfg.nc):
        sA = slotA[c].reshape(nAt, P)
        sB = slotB[c].reshape(nBt, P)
        scols = np.empty((ntiles, P), np.float32)
        for b in range(nb):
            t0 = int(cumTA[b] + cumTB[b])
            scols[t0:t0 + (cumTA[b + 1] - cumTA[b])] = \
                sA[cumTA[b]:cumTA[b + 1]]
            scols[t0 + (cumTA[b + 1] - cumTA[b]):
                  t0 + (cumTA[b + 1] - cumTA[b]) + (cumTB[b + 1] - cumTB[b])] \
                = sB[cumTB[b]:cumTB[b + 1]]
        per_core.append({
            "idxA": _wrap16(idxA[c]) if nAt else
            np.zeros((P, 8), np.int16),
            "idxB": _wrap16(idxB[c]) if nBt else
            np.zeros((P, 8), np.int16),
            "slotp": scols.T.copy().astype(ml_dtypes.bfloat16),
        })
    return TA, TB, per_core


def _wrap_cols(vec, nblk, npad):
    """[npad] -> [128, nblk] with [p, b] = vec[b*128+p]."""
    v = np.zeros(npad, np.float32)
    v[: len(vec)] = vec
    return v.reshape(nblk, P).T.copy()


# ------------------------------------------------------------- device build


def build_launch(cfg, mode, TA, TB, has_bpre=False):
    """mode 1: out = relu(u' @ WA + b1 [+ rank1]) * dinv   (writes g1, bf16)
    mode 2: out = relu(v' @ W2 + b2) @ W_post + b_post     (writes y, bf16)
    """
    nb, npad, d = cfg.nblk, cfg.npad, cfg.d
    cumTA = np.concatenate([[0], np.cumsum(TA)]).astype(int)
    cumTB = np.concatenate([[0], np.cumsum(TB)]).astype(int)
    nAt, nBt = int(cumTA[-1]), int(cumTB[-1])
    ntiles = nAt + nBt
    tmaxP = max(int((TA + TB).max()), 1)

    nc = bacc.Bacc("TRN2", target_bir_lowering=False, debug=False,
                   num_devices=cfg.nc, num_swdge_queues=NSWQ)

    tablo = nc.dram_tensor("tablo", [cfg.split, d], gdt, kind="ExternalInput")
    loctab = nc.dram_tensor("loctab", [npad, d], gdt, kind="ExternalInput")
    tabhi = nc.dram_tensor("tabhi", [cfg.nhi, d], gdt, kind="ExternalInput")
    idxA_d = nc.dram_tensor("idxA", [P, max(nAt, 1) * 8], i16,
                            kind="ExternalInput")
    idxB_d = nc.dram_tensor("idxB", [P, max(nBt, 1) * 8], i16,
                            kind="ExternalInput")
    slotp_d = nc.dram_tensor("slotp", [P, ntiles], bf16, kind="ExternalInput")
    dinvw_d = nc.dram_tensor("dinvw", [P, nb], f32, kind="ExternalInput")
    nw = 1 if mode == 1 else 2
    w_d = [nc.dram_tensor(f"w{i}", [d, d], f32r, kind="ExternalInput")
           for i in range(nw)]
    bias_d = [nc.dram_tensor(f"bias{i}", [P, d // P], f32, kind="ExternalInput")
              for i in range(nw)]
    if has_bpre:
        c1rep_d = nc.dram_tensor("c1rep", [P, npad], f32, kind="ExternalInput")
        v1w_d = nc.dram_tensor("v1w", [P, d // P], f32, kind="ExternalInput")
    out_d = nc.dram_tensor("out", [npad, d], bf16, kind="ExternalOutput")
    if DEBUG_U:
        dbg_d = nc.dram_tensor("dbg", [npad, d], f32, kind="ExternalOutput")

    kd = d // P  # feature k-tiles (2)
    nsl = (npad + cfg.dense_n - 1) // cfg.dense_n
    blk_per_sl = cfg.dense_n // P

    with tile.TileContext(nc) as tc:
        with (
            tc.tile_pool(name="const", bufs=1) as cpool,
            tc.tile_pool(name="gch", bufs=6) as gpool,
            tc.tile_pool(name="loc", bufs=4) as locpool,
            tc.tile_pool(name="pmat", bufs=3) as ppool,
            tc.tile_pool(name="work", bufs=3) as wpool,
            tc.tile_pool(name="stage", bufs=3) as stpool,
            tc.tile_pool(name="zslab", bufs=2) as zpool,
            tc.tile_pool(name="uslab", bufs=4) as upool,
            tc.tile_pool(name="apsum", bufs=4, space="PSUM") as apsum,
            tc.tile_pool(name="trpsum", bufs=2, space="PSUM") as trpsum,
            tc.tile_pool(name="dpsum", bufs=2, space="PSUM") as dpsum,
        ):
            # ---- constants
            idxA_t = cpool.tile([P, max(nAt, 1) * 8], i16)
            nc.sync.dma_start(idxA_t[:], idxA_d[:])
            idxB_t = cpool.tile([P, max(nBt, 1) * 8], i16)
            nc.sync.dma_start(idxB_t[:], idxB_d[:])
            slotp_t = cpool.tile([P, ntiles], bf16)
            nc.sync.dma_start(slotp_t[:], slotp_d[:])
            iota_i = cpool.tile([P, P], i32)
            nc.gpsimd.iota(iota_i[:], pattern=[[1, P]], base=0,
                           channel_multiplier=0)
            iota_h = cpool.tile([P, P], bf16)
            nc.vector.tensor_copy(iota_h[:], iota_i[:])
            ident = cpool.tile([P, P], f32)
            make_identity(nc, ident[:])
            ident_g = cpool.tile([P, P], gdt)
            nc.vector.tensor_copy(ident_g[:], ident[:])
            dinvw_t = cpool.tile([P, nb], f32)
            nc.sync.dma_start(dinvw_t[:], dinvw_d[:])
            w_t = []  # [stage][k][m] -> [128,128] f32r lhsT tiles
            for i in range(nw):
                tiles = []
                for k in range(kd):
                    row = []
                    for m in range(kd):
                        wt = cpool.tile([P, P], f32r, name=f"wt{i}_{k}_{m}",
                                        tag=f"wt{i}_{k}_{m}")
                        nc.sync.dma_start(
                            wt[:], w_d[i][k * P:(k + 1) * P, m * P:(m + 1) * P])
                        row.append(wt)
                    tiles.append(row)
                w_t.append(tiles)
            bias_t = []
            for i in range(nw):
                bt = cpool.tile([P, kd], f32, name=f"bt{i}", tag=f"bt{i}")
                nc.sync.dma_start(bt[:], bias_d[i][:])
                bias_t.append(bt)
            if has_bpre:
                c1rep_t = cpool.tile([P, npad], f32)
                nc.sync.dma_start(c1rep_t[:], c1rep_d[:])
                v1w_t = cpool.tile([P, kd], f32)
                nc.sync.dma_start(v1w_t[:], v1w_d[:])

            # ---- gather chunk machinery (flat per-plane, GCH tiles/call)
            qload = [0] * NSWQ  # greedy row-balance across SWDGE queues
            chunks = {"A": [], "B": []}
            issued = {"A": 0, "B": 0}
            plane_info = {
                "A": (tablo, idxA_t, nAt),
                "B": (tabhi, idxB_t, nBt),
            }

            def ensure_issued(plane, upto):
                tab_ap, idxp_t, ntot = plane_info[plane]
                upto = min(upto, ntot)
                while issued[plane] < upto:
                    c0 = issued[plane]
                    cn = min(GCH, ntot - c0)
                    gt = gpool.tile([P, GCH, d], gdt, tag="gch",
                                    name=f"g{plane}_{c0}")
                    q = qload.index(min(qload))
                    nc.gpsimd.dma_gather(
                        out_ap=gt[:, 0:cn, :], in_ap=tab_ap[:],
                        idxs_ap=idxp_t[:, c0 * 8:(c0 + cn) * 8],
                        num_idxs=cn * P, num_idxs_reg=cn * P, elem_size=d,
                        queue_num=q)
                    qload[q] += cn
                    chunks[plane].append(gt)
                    issued[plane] += cn

            def gtile(plane, t):
                return chunks[plane][t // GCH][:, t % GCH, :]

            # feature-major activation slabs, ring-buffered per dense slice
            uT_s = [None] * nsl

            def dense_slice(sl):
                s0 = sl * cfg.dense_n
                ns = min(cfg.dense_n, npad - s0)
                pz = [dpsum.tile([P, ns], f32, space="PSUM", tag="dps",
                                 name=f"pz{sl}_{dt}") for dt in range(kd)]
                for dt in range(kd):
                    for m in range(kd):
                        nc.tensor.matmul(
                            pz[dt][:], lhsT=w_t[0][m][dt][:],
                            rhs=uT_s[sl][:, m, 0:ns],
                            start=(m == 0), stop=(m == kd - 1))
                if has_bpre:
                    for dt in range(kd):
                        tmp = wpool.tile([P, cfg.dense_n], f32, tag="r1")
                        nc.vector.tensor_scalar_mul(
                            tmp[:, 0:ns], c1rep_t[:, s0:s0 + ns],
                            v1w_t[:, dt:dt + 1])
                        nc.vector.tensor_tensor(
                            out=pz[dt][:], in0=pz[dt][:], in1=tmp[:, 0:ns],
                            op=mybir.AluOpType.add)

                if mode == 1:
                    final = zpool.tile([P, kd, cfg.dense_n], f32, tag="zr")
                    for dt in range(kd):
                        nc.scalar.activation(
                            final[:, dt, 0:ns], pz[dt][:],
                            mybir.ActivationFunctionType.Relu,
                            bias=bias_t[0][:, dt:dt + 1], scale=1.0)
                else:
                    rT = zpool.tile([P, kd, cfg.dense_n], f32r, tag="zr")
                    for dt in range(kd):
                        nc.scalar.activation(
                            rT[:, dt, 0:ns], pz[dt][:],
                            mybir.ActivationFunctionType.Relu,
                            bias=bias_t[0][:, dt:dt + 1], scale=1.0)
                    py = [dpsum.tile([P, ns], f32, space="PSUM", tag="dps",
                                     name=f"py{sl}_{dt}") for dt in range(kd)]
                    for dt in range(kd):
                        for m in range(kd):
                            nc.tensor.matmul(
                                py[dt][:], lhsT=w_t[1][m][dt][:],
                                rhs=rT[:, m, 0:ns],
                                start=(m == 0), stop=(m == kd - 1))
                    final = zpool.tile([P, kd, cfg.dense_n], f32, tag="yT")
                    for dt in range(kd):
                        nc.scalar.activation(
                            final[:, dt, 0:ns], py[dt][:],
                            mybir.ActivationFunctionType.Identity,
                            bias=bias_t[1][:, dt:dt + 1], scale=1.0)

                for jj in range(ns // P):
                    blkj = (s0 + jj * P) // P
                    ost = stpool.tile([P, d], bf16, tag="ost")
                    for dt in range(kd):
                        ptr2 = trpsum.tile([P, P], f32, space="PSUM",
                                           tag="ptr")
                        nc.tensor.transpose(
                            out=ptr2[:], in_=final[:, dt, jj * P:(jj + 1) * P],
                            identity=ident[:])
                        nc.vector.tensor_copy(
                            ost[:, dt * P:(dt + 1) * P], ptr2[:])
                    nc.sync.dma_start(out_d[blkj * P:(blkj + 1) * P, :],
                                      ost[:])

            # ---- aggregation pass
            for b in range(nb):
                ta, tb = int(TA[b]), int(TB[b])
                tbt = ta + tb
                tcol = int(cumTA[b] + cumTB[b])
                bahead = min(b + LOOK, nb)
                ensure_issued("A", int(cumTA[bahead]))
                ensure_issued("B", int(cumTB[bahead]))

                psum_a = apsum.tile([P, d], f32, space="PSUM", tag="psum_a")
                if tbt:
                    p_all = ppool.tile([P, tmaxP, P], bf16, tag="pmat")
                    nc.vector.tensor_tensor(
                        out=p_all[:, 0:tbt, :],
                        in0=slotp_t[:, tcol:tcol + tbt, None].to_broadcast(
                            [P, tbt, P]),
                        in1=iota_h[:, None, :].to_broadcast([P, tbt, P]),
                        op=mybir.AluOpType.is_equal)
                selft = locpool.tile([P, d], gdt, tag="selft")
                nc.sync.dma_start(selft[:], loctab[b * P:(b + 1) * P, :])
                nc.tensor.matmul(psum_a[:], lhsT=ident_g[:], rhs=selft[:],
                                 start=True, stop=(tbt == 0))
                j = 0
                for t in range(ta):
                    nc.tensor.matmul(
                        psum_a[:], lhsT=p_all[:, j, :],
                        rhs=gtile("A", int(cumTA[b]) + t),
                        start=False, stop=(j == tbt - 1))
                    j += 1
                for t in range(tb):
                    nc.tensor.matmul(
                        psum_a[:], lhsT=p_all[:, j, :],
                        rhs=gtile("B", int(cumTB[b]) + t),
                        start=False, stop=(j == tbt - 1))
                    j += 1

                # epilogue: u' = psum * dinv (self-loops via loctab matmul)
                sl, off = divmod(b * P, cfg.dense_n)
                if off == 0:
                    uT_s[sl] = upool.tile(
                        [P, kd, min(cfg.dense_n, npad - sl * cfg.dense_n)],
                        f32r, tag="uslab", name=f"uTs{sl}")
                u2 = wpool.tile([P, d], f32, tag="u2")
                nc.scalar.mul(u2[:], psum_a[:], dinvw_t[:, b:b + 1])
                if DEBUG_U:
                    nc.sync.dma_start(dbg_d[b * P:(b + 1) * P, :], u2[:])
                for m in range(kd):
                    ptr = trpsum.tile([P, P], f32, space="PSUM", tag="ptr")
                    nc.tensor.transpose(out=ptr[:], in_=u2[:, m * P:(m + 1) * P],
                                        identity=ident[:])
                    nc.vector.tensor_copy(uT_s[sl][:, m, off:off + P], ptr[:])

                if DENSE_INTERLEAVE and (b == nb - 1
                                         or (b + 1) % blk_per_sl == 0):
                    dense_slice(sl)
            if not DENSE_INTERLEAVE:
                for sl2 in range(nsl):
                    dense_slice(sl2)

    nc.compile()
    return nc


# ------------------------------------------------------------------ driver


def _run(cfg, nc_prog, per_core_common, per_core_vars, trace=False):
    in_maps = []
    for c in range(cfg.nc):
        m = dict(per_core_common)
        m.update(per_core_vars[c])
        in_maps.append(m)
    res = run_bass_kernel_spmd(nc_prog, in_maps, core_ids=list(range(cfg.nc)),
                               trace=trace)
    return res


def gcn_forward(cfg, x, edge_index, W_pre, b_pre, W1, b1, W2, b2, W_post,
                b_post, trace=False, ret_times=None):
    x = np.asarray(x, np.float32)
    src = np.asarray(edge_index[0], np.int64)
    dst = np.asarray(edge_index[1], np.int64)
    W_pre, W1, W2, W_post = (np.asarray(w, np.float32)
                             for w in (W_pre, W1, W2, W_post))
    b_pre, b1, b2, b_post = (np.asarray(b, np.float32)
                             for b in (b_pre, b1, b2, b_post))

    n, d, nl, nb, npad = cfg.n_nodes, cfg.d, cfg.nloc, cfg.nblk, cfg.npad
    deg = (np.bincount(dst, minlength=n) + 1).astype(np.float64)
    dinv = (1.0 / np.sqrt(deg)).astype(np.float32)

    TA, TB, edge_planes = _prep_edges(cfg, src, dst)

    def local_pad(tab, c):
        out = np.zeros((npad, d), tab.dtype)
        out[:nl] = tab[c * nl:(c + 1) * nl]
        return out

    xs = x * dinv[:, None]
    WA = (W_pre.astype(np.float64) @ W1.astype(np.float64)).astype(np.float32)

    has_bpre = bool(np.any(b_pre != 0))
    dinv_cols = [
        _wrap_cols(dinv[c * nl:(c + 1) * nl], nb, npad) for c in range(cfg.nc)]

    # ---------- launch 1
    prog1 = build_launch(cfg, 1, TA, TB, has_bpre=has_bpre)
    tdt = ml_dtypes.bfloat16
    common1 = {
        "tablo": xs[: cfg.split].astype(tdt),
        "tabhi": xs[cfg.n_nodes - cfg.nhi:].astype(tdt),
        "w0": WA,
        "bias0": b1.reshape(d // P, P).T.copy(),
    }
    if has_bpre:
        v1 = (b_pre.astype(np.float64) @ W1.astype(np.float64)).astype(
            np.float32)
        common1["v1w"] = v1.reshape(d // P, P).T.copy()
        # c1[dst] = (s[dst] + dinv[dst]) * dinv[dst],  s = sum_e dinv[src]
        s = np.zeros(n, np.float64)
        np.add.at(s, dst, dinv[src].astype(np.float64))
        c1_full = ((s + dinv) * dinv).astype(np.float32)
    vars1 = []
    for c in range(cfg.nc):
        v = {
            "loctab": local_pad(xs.astype(tdt), c),
            "idxA": edge_planes[c]["idxA"],
            "idxB": edge_planes[c]["idxB"],
            "slotp": edge_planes[c]["slotp"],
            "dinvw": dinv_cols[c],
        }
        if has_bpre:
            cl = np.zeros(npad, np.float32)
            cl[:nl] = c1_full[c * nl:(c + 1) * nl]
            v["c1rep"] = np.tile(cl, (P, 1))
        vars1.append(v)
    res1 = _run(cfg, prog1, common1, vars1, trace=trace)
    g1 = np.concatenate(
        [res1.results[c]["out"][:nl].astype(np.float32)
         for c in range(cfg.nc)])
    g1 *= dinv[:, None]
    if ret_times is not None:
        ret_times.append(res1.exec_time_ns)

    # ---------- launch 2
    prog2 = build_launch(cfg, 2, TA, TB, has_bpre=False)
    common2 = {
        "tablo": g1[: cfg.split].astype(tdt),
        "tabhi": g1[cfg.n_nodes - cfg.nhi:].astype(tdt),
        "w0": W2,
        "w1": W_post,
        "bias0": b2.reshape(d // P, P).T.copy(),
        "bias1": b_post.reshape(d // P, P).T.copy(),
    }
    vars2 = []
    for c in range(cfg.nc):
        vars2.append({
            "loctab": local_pad(g1.astype(tdt), c),
            "idxA": edge_planes[c]["idxA"],
            "idxB": edge_planes[c]["idxB"],
            "slotp": edge_planes[c]["slotp"],
            "dinvw": dinv_cols[c],
        })
    res2 = _run(cfg, prog2, common2, vars2, trace=trace)
    y = np.concatenate(
        [res2.results[c]["out"][:nl].astype(np.float32)
         for c in range(cfg.nc)])
    if ret_times is not None:
        ret_times.append(res2.exec_time_ns)
    return y


def kernel(x, edge_index, W_pre, b_pre, W1, b1, W2, b2, W_post, b_post):
    cfg = Cfg()
    return gcn_forward(cfg, x, edge_index, W_pre, b_pre, W1, b1, W2, b2,
                       W_post, b_post)


# revision 13
# speedup vs baseline: 1.1873x; 1.1873x over previous
"""GCN NodeAttributeAggregator on 8 Trainium2 NeuronCores.

Strategy (node-sharded, dst-partitioned edges):
  - Host precomputes index metadata: degrees (with self-loops), dinv=rsqrt(deg),
    per-core edge lists partitioned by dst owner, grouped by (dst-block of 128,
    src<32768 vs src>=32768 for int16 gather indices), padded to 128-edge tiles.
  - Device per core: dma_gather rows of a pre-scaled node table (xs = x*dinv),
    scatter-by-matmul: one-hot P matrices (built on DVE via iota + is_equal)
    contract each 128-edge tile into a 128-dst PSUM accumulator.
  - Gathers are issued as flat GCH-tile chunks per table plane (A = rows
    [0, split), B = last nhi rows), spanning dst-block boundaries, so SWDGE
    descriptor-generation on the Pool engine amortizes its fixed cost and the
    DMA engines stay ahead of the PE.
  - Dense 256x256 matmuls run in float32r (full PE rate) feature-major, with
    PE transposes at layout boundaries; the dense pass is interleaved every
    4 dst-blocks so stores overlap the aggregation stream.
  - Algebra: GCN layer out = D^-1/2 (A+I) D^-1/2 h W.  Row scaling commutes
    with right matmuls, relu commutes with positive row scaling, and
    agg(h W) = agg(h) W, so:
      L1 (fused W_pre@W1): u' = (scatter(xs) + xs_dst) * dinv;
          g1 = relu(u' @ (W_pre W1) + b1 + rank1(b_pre)) * dinv
      L2: v' = (scatter(g1) + g1_dst) * dinv;
          y  = relu(v' @ W2 + b2) @ W_post + b_post
  - Two SPMD launches; host concatenates g1 slices between them.  Both
    launches write bf16 outputs (g1 is re-quantized to bf16 tables anyway).
"""

import dataclasses
import ml_dtypes
import numpy as np

import concourse.bacc as bacc
import concourse.bass as bass
import concourse.tile as tile
import concourse.mybir as mybir
from concourse.bass_utils import run_bass_kernel_spmd
from concourse.masks import make_identity

P = 128
NSWQ = 4  # SWDGE queues (ucode max 4)
GCH = 8  # gather tiles per dma_gather call (HW ring: 1024 descs/queue)
LOOK = 4  # blocks of gather lookahead
DENSE_INTERLEAVE = True
DEBUG_U = False
f32 = mybir.dt.float32
f32r = mybir.dt.float32r
bf16 = mybir.dt.bfloat16
gdt = bf16
i16 = mybir.dt.int16
i32 = mybir.dt.int32


@dataclasses.dataclass
class Cfg:
    n_nodes: int = 50000
    d: int = 256
    nc: int = 8
    split: int = 32768
    dense_n: int = 512

    @property
    def nloc(self):
        return self.n_nodes // self.nc

    @property
    def nblk(self):
        return (self.nloc + P - 1) // P

    @property
    def npad(self):
        return self.nblk * P

    @property
    def nhi(self):
        # table B spans the last min(32768, n) rows (full int16 window)
        return min(self.split, self.n_nodes)


# ---------------------------------------------------------------- host prep


def _wrap16(vals):
    """[n] -> [128, n//16] int16: value i at [i%16, i//16], replicated x8."""
    w = vals.reshape(-1, 16).T
    return np.tile(w, (8, 1)).astype(np.int16)


def _prep_edges(cfg, src, dst):
    """Partition edges by dst owner; split per dst-block into two gather
    groups (table A = rows [0, split); table B = rows [baseB, n)).  The cut
    is chosen per block at an exact multiple of 128 edges in src-order when
    the overlap window [baseB, split) allows, so group A has no padding.

    Emits per-core flat gather planes (idxA: all A tiles block-major,
    idxB likewise) plus a merged slot plane (block-major, A tiles then B
    tiles within a block) with identical compile-time schedule (TA, TB)
    across cores.
    """
    nl, nb = cfg.nloc, cfg.nblk
    baseB = cfg.n_nodes - cfg.nhi
    owner = dst // nl
    loc = dst - owner * nl
    blk = loc // P
    slot = loc - blk * P

    key = owner * nb + blk
    nkeys = cfg.nc * nb
    n_cb = np.bincount(key, minlength=nkeys).reshape(cfg.nc, nb)
    cntlo = np.bincount(key[src < baseB], minlength=nkeys).reshape(cfg.nc, nb)
    cntA = np.bincount(key[src < cfg.split], minlength=nkeys).reshape(cfg.nc, nb)

    lo = ((cntlo + P - 1) // P).max(axis=0)  # [nb] min feasible TA
    hi = (cntA // P).min(axis=0)             # [nb] max feasible TA
    feasible = lo <= hi
    # prefer max A (A has no pad); fallback: threshold split at `split`
    TA = np.where(feasible, hi, ((cntA + P - 1) // P).max(axis=0))
    # B count per (c,b): n - A_count; A_count = TA*P if feasible else cntA
    A_cnt = np.where(feasible[None, :], np.minimum(TA[None, :] * P, n_cb),
                     cntA)
    B_cnt = n_cb - A_cnt
    TB = ((B_cnt + P - 1) // P).max(axis=0)

    cumTA = np.concatenate([[0], np.cumsum(TA)])
    cumTB = np.concatenate([[0], np.cumsum(TB)])
    nAt, nBt = int(cumTA[-1]), int(cumTB[-1])

    # rank edges within (core, block) by src (stable) to apply the cut
    order = np.lexsort((src, key))
    skey = key[order]
    group_start = np.concatenate(
        [[0], np.cumsum(np.bincount(skey, minlength=nkeys))])
    rank = np.arange(len(src)) - group_start[skey]

    ocore = owner[order]
    oblk = blk[order]
    acut = A_cnt[ocore, oblk]
    in_a = rank < acut
    rowpos = np.where(in_a, rank, rank - acut)
    rowsA = cumTA[oblk] * P + rowpos
    rowsB = cumTB[oblk] * P + rowpos
    idxv = (src[order] - np.where(in_a, 0, baseB)).astype(np.int16)
    slotv = slot[order].astype(np.float32)
    assert (src[order][in_a] < cfg.split).all()
    assert (src[order][~in_a] >= baseB).all()

    idxA = np.zeros((cfg.nc, nAt * P), np.int16)
    idxB = np.zeros((cfg.nc, nBt * P), np.int16)
    slotA = np.full((cfg.nc, nAt * P), 300.0, np.float32)
    slotB = np.full((cfg.nc, nBt * P), 300.0, np.float32)
    idxA[ocore[in_a], rowsA[in_a]] = idxv[in_a]
    idxB[ocore[~in_a], rowsB[~in_a]] = idxv[~in_a]
    slotA[ocore[in_a], rowsA[in_a]] = slotv[in_a]
    slotB[ocore[~in_a], rowsB[~in_a]] = slotv[~in_a]

    # merged slot plane: block-major, A tiles then B tiles within a block
    ntiles = nAt + nBt
    per_core = []
    for c in range(cfg.nc):
        sA = slotA[c].reshape(nAt, P)
        sB = slotB[c].reshape(nBt, P)
        scols = np.empty((ntiles, P), np.float32)
        for b in range(nb):
            t0 = int(cumTA[b] + cumTB[b])
            scols[t0:t0 + (cumTA[b + 1] - cumTA[b])] = \
                sA[cumTA[b]:cumTA[b + 1]]
            scols[t0 + (cumTA[b + 1] - cumTA[b]):
                  t0 + (cumTA[b + 1] - cumTA[b]) + (cumTB[b + 1] - cumTB[b])] \
                = sB[cumTB[b]:cumTB[b + 1]]
        per_core.append({
            "idxA": _wrap16(idxA[c]) if nAt else
            np.zeros((P, 8), np.int16),
            "idxB": _wrap16(idxB[c]) if nBt else
            np.zeros((P, 8), np.int16),
            "slotp": scols.T.copy(),
        })
    return TA, TB, per_core


def _wrap_cols(vec, nblk, npad):
    """[npad] -> [128, nblk] with [p, b] = vec[b*128+p]."""
    v = np.zeros(npad, np.float32)
    v[: len(vec)] = vec
    return v.reshape(nblk, P).T.copy()


# ------------------------------------------------------------- device build


def build_launch(cfg, mode, TA, TB, has_bpre=False):
    """mode 1: out = relu(u' @ WA + b1 [+ rank1]) * dinv   (writes g1, bf16)
    mode 2: out = relu(v' @ W2 + b2) @ W_post + b_post     (writes y, bf16)
    """
    nb, npad, d = cfg.nblk, cfg.npad, cfg.d
    cumTA = np.concatenate([[0], np.cumsum(TA)]).astype(int)
    cumTB = np.concatenate([[0], np.cumsum(TB)]).astype(int)
    nAt, nBt = int(cumTA[-1]), int(cumTB[-1])
    ntiles = nAt + nBt
    tmaxP = max(int((TA + TB).max()), 1)

    nc = bacc.Bacc("TRN2", target_bir_lowering=False, debug=False,
                   num_devices=cfg.nc, num_swdge_queues=NSWQ)

    tablo = nc.dram_tensor("tablo", [cfg.split, d], gdt, kind="ExternalInput")
    loctab = nc.dram_tensor("loctab", [npad, d], gdt, kind="ExternalInput")
    tabhi = nc.dram_tensor("tabhi", [cfg.nhi, d], gdt, kind="ExternalInput")
    idxA_d = nc.dram_tensor("idxA", [P, max(nAt, 1) * 8], i16,
                            kind="ExternalInput")
    idxB_d = nc.dram_tensor("idxB", [P, max(nBt, 1) * 8], i16,
                            kind="ExternalInput")
    slotp_d = nc.dram_tensor("slotp", [P, ntiles], f32, kind="ExternalInput")
    dinvw_d = nc.dram_tensor("dinvw", [P, nb], f32, kind="ExternalInput")
    nw = 1 if mode == 1 else 2
    w_d = [nc.dram_tensor(f"w{i}", [d, d], f32r, kind="ExternalInput")
           for i in range(nw)]
    bias_d = [nc.dram_tensor(f"bias{i}", [P, d // P], f32, kind="ExternalInput")
              for i in range(nw)]
    if has_bpre:
        c1rep_d = nc.dram_tensor("c1rep", [P, npad], f32, kind="ExternalInput")
        v1w_d = nc.dram_tensor("v1w", [P, d // P], f32, kind="ExternalInput")
    out_d = nc.dram_tensor("out", [npad, d], bf16, kind="ExternalOutput")
    if DEBUG_U:
        dbg_d = nc.dram_tensor("dbg", [npad, d], f32, kind="ExternalOutput")

    kd = d // P  # feature k-tiles (2)
    nsl = (npad + cfg.dense_n - 1) // cfg.dense_n
    blk_per_sl = cfg.dense_n // P

    with tile.TileContext(nc) as tc:
        with (
            tc.tile_pool(name="const", bufs=1) as cpool,
            tc.tile_pool(name="gch", bufs=12) as gpool,
            tc.tile_pool(name="loc", bufs=4) as locpool,
            tc.tile_pool(name="pmat", bufs=3) as ppool,
            tc.tile_pool(name="work", bufs=3) as wpool,
            tc.tile_pool(name="stage", bufs=3) as stpool,
            tc.tile_pool(name="zslab", bufs=2) as zpool,
            tc.tile_pool(name="uslab", bufs=4) as upool,
            tc.tile_pool(name="apsum", bufs=4, space="PSUM") as apsum,
            tc.tile_pool(name="trpsum", bufs=2, space="PSUM") as trpsum,
            tc.tile_pool(name="dpsum", bufs=2, space="PSUM") as dpsum,
        ):
            # ---- constants (idx planes first, in slices, so gathers
            # can start as soon as their columns land)
            idxA_t = cpool.tile([P, max(nAt, 1) * 8], i16)
            idxB_t = cpool.tile([P, max(nBt, 1) * 8], i16)
            slotp_t = cpool.tile([P, ntiles], f32)
            ISL = 64 * 8  # tiles' worth of idx columns per DMA slice
            for t_, d_ in ((idxA_t, idxA_d), (idxB_t, idxB_d)):
                ncols = t_.shape[1]
                for c0_ in range(0, ncols, ISL):
                    c1_ = min(c0_ + ISL, ncols)
                    nc.sync.dma_start(t_[:, c0_:c1_], d_[:, c0_:c1_])
            nc.sync.dma_start(slotp_t[:], slotp_d[:])
            iota_i = cpool.tile([P, P], i32)
            nc.gpsimd.iota(iota_i[:], pattern=[[1, P]], base=0,
                           channel_multiplier=0)
            iota_f = cpool.tile([P, P], f32)
            nc.vector.tensor_copy(iota_f[:], iota_i[:])
            ident = cpool.tile([P, P], f32)
            make_identity(nc, ident[:])
            ident_g = cpool.tile([P, P], gdt)
            nc.vector.tensor_copy(ident_g[:], ident[:])
            dinvw_t = cpool.tile([P, nb], f32)
            nc.sync.dma_start(dinvw_t[:], dinvw_d[:])
            w_t = []  # [stage][k][m] -> [128,128] f32r lhsT tiles
            for i in range(nw):
                tiles = []
                for k in range(kd):
                    row = []
                    for m in range(kd):
                        wt = cpool.tile([P, P], f32r, name=f"wt{i}_{k}_{m}",
                                        tag=f"wt{i}_{k}_{m}")
                        nc.sync.dma_start(
                            wt[:], w_d[i][k * P:(k + 1) * P, m * P:(m + 1) * P])
                        row.append(wt)
                    tiles.append(row)
                w_t.append(tiles)
            bias_t = []
            for i in range(nw):
                bt = cpool.tile([P, kd], f32, name=f"bt{i}", tag=f"bt{i}")
                nc.sync.dma_start(bt[:], bias_d[i][:])
                bias_t.append(bt)
            if has_bpre:
                c1rep_t = cpool.tile([P, npad], f32)
                nc.sync.dma_start(c1rep_t[:], c1rep_d[:])
                v1w_t = cpool.tile([P, kd], f32)
                nc.sync.dma_start(v1w_t[:], v1w_d[:])

            # ---- gather chunk machinery (flat per-plane, GCH tiles/call)
            qload = [0] * NSWQ  # greedy row-balance across SWDGE queues
            chunks = {"A": [], "B": []}
            issued = {"A": 0, "B": 0}
            plane_info = {
                "A": (tablo, idxA_t, nAt),
                "B": (tabhi, idxB_t, nBt),
            }

            def ensure_issued(plane, upto):
                tab_ap, idxp_t, ntot = plane_info[plane]
                upto = min(upto, ntot)
                while issued[plane] < upto:
                    c0 = issued[plane]
                    cn = min(GCH, ntot - c0)
                    gt = gpool.tile([P, GCH, d], gdt, tag="gch",
                                    name=f"g{plane}_{c0}")
                    q = qload.index(min(qload))
                    nc.gpsimd.dma_gather(
                        out_ap=gt[:, 0:cn, :], in_ap=tab_ap[:],
                        idxs_ap=idxp_t[:, c0 * 8:(c0 + cn) * 8],
                        num_idxs=cn * P, num_idxs_reg=cn * P, elem_size=d,
                        queue_num=q)
                    qload[q] += cn
                    chunks[plane].append(gt)
                    issued[plane] += cn

            def gtile(plane, t):
                return chunks[plane][t // GCH][:, t % GCH, :]

            # feature-major activation slabs, ring-buffered per dense slice
            uT_s = [None] * nsl

            def dense_slice(sl):
                s0 = sl * cfg.dense_n
                ns = min(cfg.dense_n, npad - s0)
                pz = [dpsum.tile([P, ns], f32, space="PSUM", tag="dps",
                                 name=f"pz{sl}_{dt}") for dt in range(kd)]
                for dt in range(kd):
                    for m in range(kd):
                        nc.tensor.matmul(
                            pz[dt][:], lhsT=w_t[0][m][dt][:],
                            rhs=uT_s[sl][:, m, 0:ns],
                            start=(m == 0), stop=(m == kd - 1))
                if has_bpre:
                    for dt in range(kd):
                        tmp = wpool.tile([P, cfg.dense_n], f32, tag="r1")
                        nc.vector.tensor_scalar_mul(
                            tmp[:, 0:ns], c1rep_t[:, s0:s0 + ns],
                            v1w_t[:, dt:dt + 1])
                        nc.vector.tensor_tensor(
                            out=pz[dt][:], in0=pz[dt][:], in1=tmp[:, 0:ns],
                            op=mybir.AluOpType.add)

                if mode == 1:
                    final = zpool.tile([P, kd, cfg.dense_n], f32, tag="zr")
                    for dt in range(kd):
                        nc.scalar.activation(
                            final[:, dt, 0:ns], pz[dt][:],
                            mybir.ActivationFunctionType.Relu,
                            bias=bias_t[0][:, dt:dt + 1], scale=1.0)
                else:
                    rT = zpool.tile([P, kd, cfg.dense_n], f32r, tag="zr")
                    for dt in range(kd):
                        nc.scalar.activation(
                            rT[:, dt, 0:ns], pz[dt][:],
                            mybir.ActivationFunctionType.Relu,
                            bias=bias_t[0][:, dt:dt + 1], scale=1.0)
                    py = [dpsum.tile([P, ns], f32, space="PSUM", tag="dps",
                                     name=f"py{sl}_{dt}") for dt in range(kd)]
                    for dt in range(kd):
                        for m in range(kd):
                            nc.tensor.matmul(
                                py[dt][:], lhsT=w_t[1][m][dt][:],
                                rhs=rT[:, m, 0:ns],
                                start=(m == 0), stop=(m == kd - 1))
                    final = zpool.tile([P, kd, cfg.dense_n], f32, tag="yT")
                    for dt in range(kd):
                        nc.scalar.activation(
                            final[:, dt, 0:ns], py[dt][:],
                            mybir.ActivationFunctionType.Identity,
                            bias=bias_t[1][:, dt:dt + 1], scale=1.0)

                for jj in range(ns // P):
                    blkj = (s0 + jj * P) // P
                    ost = stpool.tile([P, d], bf16, tag="ost")
                    for dt in range(kd):
                        ptr2 = trpsum.tile([P, P], f32, space="PSUM",
                                           tag="ptr")
                        nc.tensor.transpose(
                            out=ptr2[:], in_=final[:, dt, jj * P:(jj + 1) * P],
                            identity=ident[:])
                        nc.vector.tensor_copy(
                            ost[:, dt * P:(dt + 1) * P], ptr2[:])
                    nc.sync.dma_start(out_d[blkj * P:(blkj + 1) * P, :],
                                      ost[:])

            # ---- aggregation pass
            for b in range(nb):
                ta, tb = int(TA[b]), int(TB[b])
                tbt = ta + tb
                tcol = int(cumTA[b] + cumTB[b])
                bahead = min(b + LOOK, nb)
                ensure_issued("A", int(cumTA[bahead]))
                ensure_issued("B", int(cumTB[bahead]))

                psum_a = apsum.tile([P, d], f32, space="PSUM", tag="psum_a")
                if tbt:
                    p_all = ppool.tile([P, tmaxP, P], bf16, tag="pmat")
                    nc.vector.tensor_tensor(
                        out=p_all[:, 0:tbt, :],
                        in0=slotp_t[:, tcol:tcol + tbt, None].to_broadcast(
                            [P, tbt, P]),
                        in1=iota_f[:, None, :].to_broadcast([P, tbt, P]),
                        op=mybir.AluOpType.is_equal)
                selft = locpool.tile([P, d], gdt, tag="selft")
                nc.sync.dma_start(selft[:], loctab[b * P:(b + 1) * P, :])
                nc.tensor.matmul(psum_a[:], lhsT=ident_g[:], rhs=selft[:],
                                 start=True, stop=(tbt == 0))
                j = 0
                for t in range(ta):
                    nc.tensor.matmul(
                        psum_a[:], lhsT=p_all[:, j, :],
                        rhs=gtile("A", int(cumTA[b]) + t),
                        start=False, stop=(j == tbt - 1))
                    j += 1
                for t in range(tb):
                    nc.tensor.matmul(
                        psum_a[:], lhsT=p_all[:, j, :],
                        rhs=gtile("B", int(cumTB[b]) + t),
                        start=False, stop=(j == tbt - 1))
                    j += 1

                # epilogue: u' = psum * dinv (self-loops via loctab matmul)
                sl, off = divmod(b * P, cfg.dense_n)
                if off == 0:
                    uT_s[sl] = upool.tile(
                        [P, kd, min(cfg.dense_n, npad - sl * cfg.dense_n)],
                        f32r, tag="uslab", name=f"uTs{sl}")
                u2 = wpool.tile([P, d], f32, tag="u2")
                nc.scalar.mul(u2[:], psum_a[:], dinvw_t[:, b:b + 1])
                if DEBUG_U:
                    nc.sync.dma_start(dbg_d[b * P:(b + 1) * P, :], u2[:])
                for m in range(kd):
                    ptr = trpsum.tile([P, P], f32, space="PSUM", tag="ptr")
                    nc.tensor.transpose(out=ptr[:], in_=u2[:, m * P:(m + 1) * P],
                                        identity=ident[:])
                    nc.vector.tensor_copy(uT_s[sl][:, m, off:off + P], ptr[:])

                if DENSE_INTERLEAVE and (b == nb - 1
                                         or (b + 1) % blk_per_sl == 0):
                    dense_slice(sl)
            if not DENSE_INTERLEAVE:
                for sl2 in range(nsl):
                    dense_slice(sl2)

    nc.compile()
    return nc


# ------------------------------------------------------------------ driver


def _run(cfg, nc_prog, per_core_common, per_core_vars, trace=False):
    in_maps = []
    for c in range(cfg.nc):
        m = dict(per_core_common)
        m.update(per_core_vars[c])
        in_maps.append(m)
    res = run_bass_kernel_spmd(nc_prog, in_maps, core_ids=list(range(cfg.nc)),
                               trace=trace)
    return res


def gcn_forward(cfg, x, edge_index, W_pre, b_pre, W1, b1, W2, b2, W_post,
                b_post, trace=False, ret_times=None):
    x = np.asarray(x, np.float32)
    src = np.asarray(edge_index[0], np.int64)
    dst = np.asarray(edge_index[1], np.int64)
    W_pre, W1, W2, W_post = (np.asarray(w, np.float32)
                             for w in (W_pre, W1, W2, W_post))
    b_pre, b1, b2, b_post = (np.asarray(b, np.float32)
                             for b in (b_pre, b1, b2, b_post))

    n, d, nl, nb, npad = cfg.n_nodes, cfg.d, cfg.nloc, cfg.nblk, cfg.npad
    deg = (np.bincount(dst, minlength=n) + 1).astype(np.float64)
    dinv = (1.0 / np.sqrt(deg)).astype(np.float32)

    TA, TB, edge_planes = _prep_edges(cfg, src, dst)

    def local_pad(tab, c):
        out = np.zeros((npad, d), tab.dtype)
        out[:nl] = tab[c * nl:(c + 1) * nl]
        return out

    xs = x * dinv[:, None]
    WA = (W_pre.astype(np.float64) @ W1.astype(np.float64)).astype(np.float32)

    has_bpre = bool(np.any(b_pre != 0))
    dinv_cols = [
        _wrap_cols(dinv[c * nl:(c + 1) * nl], nb, npad) for c in range(cfg.nc)]

    # ---------- launch 1
    prog1 = build_launch(cfg, 1, TA, TB, has_bpre=has_bpre)
    tdt = ml_dtypes.bfloat16
    common1 = {
        "tablo": xs[: cfg.split].astype(tdt),
        "tabhi": xs[cfg.n_nodes - cfg.nhi:].astype(tdt),
        "w0": WA,
        "bias0": b1.reshape(d // P, P).T.copy(),
    }
    if has_bpre:
        v1 = (b_pre.astype(np.float64) @ W1.astype(np.float64)).astype(
            np.float32)
        common1["v1w"] = v1.reshape(d // P, P).T.copy()
        # c1[dst] = (s[dst] + dinv[dst]) * dinv[dst],  s = sum_e dinv[src]
        s = np.zeros(n, np.float64)
        np.add.at(s, dst, dinv[src].astype(np.float64))
        c1_full = ((s + dinv) * dinv).astype(np.float32)
    vars1 = []
    for c in range(cfg.nc):
        v = {
            "loctab": local_pad(xs.astype(tdt), c),
            "idxA": edge_planes[c]["idxA"],
            "idxB": edge_planes[c]["idxB"],
            "slotp": edge_planes[c]["slotp"],
            "dinvw": dinv_cols[c],
        }
        if has_bpre:
            cl = np.zeros(npad, np.float32)
            cl[:nl] = c1_full[c * nl:(c + 1) * nl]
            v["c1rep"] = np.tile(cl, (P, 1))
        vars1.append(v)
    res1 = _run(cfg, prog1, common1, vars1, trace=trace)
    g1 = np.concatenate(
        [res1.results[c]["out"][:nl].astype(np.float32)
         for c in range(cfg.nc)])
    g1 *= dinv[:, None]
    if ret_times is not None:
        ret_times.append(res1.exec_time_ns)

    # ---------- launch 2
    prog2 = build_launch(cfg, 2, TA, TB, has_bpre=False)
    common2 = {
        "tablo": g1[: cfg.split].astype(tdt),
        "tabhi": g1[cfg.n_nodes - cfg.nhi:].astype(tdt),
        "w0": W2,
        "w1": W_post,
        "bias0": b2.reshape(d // P, P).T.copy(),
        "bias1": b_post.reshape(d // P, P).T.copy(),
    }
    vars2 = []
    for c in range(cfg.nc):
        vars2.append({
            "loctab": local_pad(g1.astype(tdt), c),
            "idxA": edge_planes[c]["idxA"],
            "idxB": edge_planes[c]["idxB"],
            "slotp": edge_planes[c]["slotp"],
            "dinvw": dinv_cols[c],
        })
    res2 = _run(cfg, prog2, common2, vars2, trace=trace)
    y = np.concatenate(
        [res2.results[c]["out"][:nl].astype(np.float32)
         for c in range(cfg.nc)])
    if ret_times is not None:
        ret_times.append(res2.exec_time_ns)
    return y


def kernel(x, edge_index, W_pre, b_pre, W1, b1, W2, b2, W_post, b_post):
    cfg = Cfg()
    return gcn_forward(cfg, x, edge_index, W_pre, b_pre, W1, b1, W2, b2,
                       W_post, b_post)


# revision 14
# speedup vs baseline: 1.4449x; 1.2170x over previous
"""GCN NodeAttributeAggregator on 8 Trainium2 NeuronCores.

Strategy (node-sharded, dst-partitioned edges):
  - Host precomputes index metadata: degrees (with self-loops), dinv=rsqrt(deg),
    per-core edge lists partitioned by dst owner, grouped by (dst-block of 128,
    src<32768 vs src>=32768 for int16 gather indices), padded to 128-edge tiles.
  - Device per core: dma_gather rows of a pre-scaled node table (xs = x*dinv),
    scatter-by-matmul: one-hot P matrices (built on DVE via iota + is_equal)
    contract each 128-edge tile into a 128-dst PSUM accumulator.
  - Gathers are issued as flat GCH-tile chunks per table plane (A = rows
    [0, split), B = last nhi rows), spanning dst-block boundaries, so SWDGE
    descriptor-generation on the Pool engine amortizes its fixed cost and the
    DMA engines stay ahead of the PE.
  - Dense 256x256 matmuls run in float32r (full PE rate) feature-major, with
    PE transposes at layout boundaries; the dense pass is interleaved every
    4 dst-blocks so stores overlap the aggregation stream.
  - Algebra: GCN layer out = D^-1/2 (A+I) D^-1/2 h W.  Row scaling commutes
    with right matmuls, relu commutes with positive row scaling, and
    agg(h W) = agg(h) W, so:
      L1 (fused W_pre@W1): u' = (scatter(xs) + xs_dst) * dinv;
          g1 = relu(u' @ (W_pre W1) + b1 + rank1(b_pre)) * dinv
      L2: v' = (scatter(g1) + g1_dst) * dinv;
          y  = relu(v' @ W2 + b2) @ W_post + b_post
  - Two SPMD launches; host concatenates g1 slices between them.  Both
    launches write bf16 outputs (g1 is re-quantized to bf16 tables anyway).
"""

import dataclasses
import ml_dtypes
import numpy as np

import concourse.bacc as bacc
import concourse.bass as bass
import concourse.tile as tile
import concourse.mybir as mybir
from concourse.bass_utils import run_bass_kernel_spmd
from concourse.masks import make_identity

P = 128
NSWQ = 4  # SWDGE queues (ucode max 4)
GCH = 8  # gather tiles per dma_gather call (HW ring: 1024 descs/queue)
LOOK = 4  # blocks of gather lookahead
DENSE_INTERLEAVE = True
DEBUG_U = False
f32 = mybir.dt.float32
f32r = mybir.dt.float32r
bf16 = mybir.dt.bfloat16
gdt = bf16
i16 = mybir.dt.int16
i32 = mybir.dt.int32


@dataclasses.dataclass
class Cfg:
    n_nodes: int = 50000
    d: int = 256
    nc: int = 8
    split: int = 32768
    dense_n: int = 512

    @property
    def nloc(self):
        return self.n_nodes // self.nc

    @property
    def nblk(self):
        return (self.nloc + P - 1) // P

    @property
    def npad(self):
        return self.nblk * P

    @property
    def nhi(self):
        # table B spans the last min(32768, n) rows (full int16 window)
        return min(self.split, self.n_nodes)


# ---------------------------------------------------------------- host prep


def _wrap16(vals):
    """[n] -> [128, n//16] int16: value i at [i%16, i//16], replicated x8."""
    w = vals.reshape(-1, 16).T
    return np.tile(w, (8, 1)).astype(np.int16)


def _prep_edges(cfg, src, dst):
    """Partition edges by dst owner; split per dst-block into two gather
    groups (table A = rows [0, split); table B = rows [baseB, n)).  The cut
    is chosen per block at an exact multiple of 128 edges in src-order when
    the overlap window [baseB, split) allows, so group A has no padding.

    Emits per-core flat gather planes (idxA: all A tiles block-major,
    idxB likewise) plus a merged slot plane (block-major, A tiles then B
    tiles within a block) with identical compile-time schedule (TA, TB)
    across cores.
    """
    nl, nb = cfg.nloc, cfg.nblk
    baseB = cfg.n_nodes - cfg.nhi
    owner = dst // nl
    loc = dst - owner * nl
    blk = loc // P
    slot = loc - blk * P

    key = owner * nb + blk
    nkeys = cfg.nc * nb
    n_cb = np.bincount(key, minlength=nkeys).reshape(cfg.nc, nb)
    cntlo = np.bincount(key[src < baseB], minlength=nkeys).reshape(cfg.nc, nb)
    cntA = np.bincount(key[src < cfg.split], minlength=nkeys).reshape(cfg.nc, nb)

    lo = ((cntlo + P - 1) // P).max(axis=0)  # [nb] min feasible TA
    hi = (cntA // P).min(axis=0)             # [nb] max feasible TA
    feasible = lo <= hi
    # prefer max A (A has no pad); fallback: threshold split at `split`
    TA = np.where(feasible, hi, ((cntA + P - 1) // P).max(axis=0))
    # B count per (c,b): n - A_count; A_count = TA*P if feasible else cntA
    A_cnt = np.where(feasible[None, :], np.minimum(TA[None, :] * P, n_cb),
                     cntA)
    B_cnt = n_cb - A_cnt
    TB = ((B_cnt + P - 1) // P).max(axis=0)

    cumTA = np.concatenate([[0], np.cumsum(TA)])
    cumTB = np.concatenate([[0], np.cumsum(TB)])
    nAt, nBt = int(cumTA[-1]), int(cumTB[-1])

    # rank edges within (core, block) by src (stable) to apply the cut
    order = np.lexsort((src, key))
    skey = key[order]
    group_start = np.concatenate(
        [[0], np.cumsum(np.bincount(skey, minlength=nkeys))])
    rank = np.arange(len(src)) - group_start[skey]

    ocore = owner[order]
    oblk = blk[order]
    acut = A_cnt[ocore, oblk]
    in_a = rank < acut
    rowpos = np.where(in_a, rank, rank - acut)
    rowsA = cumTA[oblk] * P + rowpos
    rowsB = cumTB[oblk] * P + rowpos
    idxv = (src[order] - np.where(in_a, 0, baseB)).astype(np.int16)
    slotv = slot[order].astype(np.float32)
    assert (src[order][in_a] < cfg.split).all()
    assert (src[order][~in_a] >= baseB).all()

    idxA = np.zeros((cfg.nc, nAt * P), np.int16)
    idxB = np.zeros((cfg.nc, nBt * P), np.int16)
    slotA = np.full((cfg.nc, nAt * P), 300.0, np.float32)
    slotB = np.full((cfg.nc, nBt * P), 300.0, np.float32)
    idxA[ocore[in_a], rowsA[in_a]] = idxv[in_a]
    idxB[ocore[~in_a], rowsB[~in_a]] = idxv[~in_a]
    slotA[ocore[in_a], rowsA[in_a]] = slotv[in_a]
    slotB[ocore[~in_a], rowsB[~in_a]] = slotv[~in_a]

    # merged slot plane: block-major, A tiles then B tiles within a block
    ntiles = nAt + nBt
    per_core = []
    for c in range(cfg.nc):
        sA = slotA[c].reshape(nAt, P)
        sB = slotB[c].reshape(nBt, P)
        scols = np.empty((ntiles, P), np.float32)
        for b in range(nb):
            t0 = int(cumTA[b] + cumTB[b])
            scols[t0:t0 + (cumTA[b + 1] - cumTA[b])] = \
                sA[cumTA[b]:cumTA[b + 1]]
            scols[t0 + (cumTA[b + 1] - cumTA[b]):
                  t0 + (cumTA[b + 1] - cumTA[b]) + (cumTB[b + 1] - cumTB[b])] \
                = sB[cumTB[b]:cumTB[b + 1]]
        per_core.append({
            "idxA": _wrap16(idxA[c]) if nAt else
            np.zeros((P, 8), np.int16),
            "idxB": _wrap16(idxB[c]) if nBt else
            np.zeros((P, 8), np.int16),
            "slotp": scols.T.copy(),
        })
    return TA, TB, per_core


def _wrap_cols(vec, nblk, npad):
    """[npad] -> [128, nblk] with [p, b] = vec[b*128+p]."""
    v = np.zeros(npad, np.float32)
    v[: len(vec)] = vec
    return v.reshape(nblk, P).T.copy()


# ------------------------------------------------------------- device build


def build_launch(cfg, mode, TA, TB, has_bpre=False):
    """mode 1: out = relu(u' @ WA + b1 [+ rank1]) * dinv   (writes g1, bf16)
    mode 2: out = relu(v' @ W2 + b2) @ W_post + b_post     (writes y, bf16)
    """
    nb, npad, d = cfg.nblk, cfg.npad, cfg.d
    cumTA = np.concatenate([[0], np.cumsum(TA)]).astype(int)
    cumTB = np.concatenate([[0], np.cumsum(TB)]).astype(int)
    nAt, nBt = int(cumTA[-1]), int(cumTB[-1])
    ntiles = nAt + nBt
    tmaxP = max(int((TA + TB).max()), 1)

    nc = bacc.Bacc("TRN2", target_bir_lowering=False, debug=False,
                   num_devices=cfg.nc, num_swdge_queues=NSWQ)

    tablo = nc.dram_tensor("tablo", [cfg.split, d], gdt, kind="ExternalInput")
    loctab = nc.dram_tensor("loctab", [npad, d], gdt, kind="ExternalInput")
    tabhi = nc.dram_tensor("tabhi", [cfg.nhi, d], gdt, kind="ExternalInput")
    idxA_d = nc.dram_tensor("idxA", [P, max(nAt, 1) * 8], i16,
                            kind="ExternalInput")
    idxB_d = nc.dram_tensor("idxB", [P, max(nBt, 1) * 8], i16,
                            kind="ExternalInput")
    slotp_d = nc.dram_tensor("slotp", [P, ntiles], f32, kind="ExternalInput")
    dinvw_d = nc.dram_tensor("dinvw", [P, nb], f32, kind="ExternalInput")
    nw = 1 if mode == 1 else 2
    w_d = [nc.dram_tensor(f"w{i}", [d, d], f32r, kind="ExternalInput")
           for i in range(nw)]
    bias_d = [nc.dram_tensor(f"bias{i}", [P, d // P], f32, kind="ExternalInput")
              for i in range(nw)]
    if has_bpre:
        c1rep_d = nc.dram_tensor("c1rep", [P, npad], f32, kind="ExternalInput")
        v1w_d = nc.dram_tensor("v1w", [P, d // P], f32, kind="ExternalInput")
    out_d = nc.dram_tensor("out", [npad, d], bf16, kind="ExternalOutput")
    if DEBUG_U:
        dbg_d = nc.dram_tensor("dbg", [npad, d], f32, kind="ExternalOutput")

    kd = d // P  # feature k-tiles (2)
    nsl = (npad + cfg.dense_n - 1) // cfg.dense_n
    blk_per_sl = cfg.dense_n // P

    with tile.TileContext(nc) as tc:
        with (
            tc.tile_pool(name="const", bufs=1) as cpool,
            tc.tile_pool(name="gch", bufs=12) as gpool,
            tc.tile_pool(name="loc", bufs=4) as locpool,
            tc.tile_pool(name="pmat", bufs=3) as ppool,
            tc.tile_pool(name="work", bufs=3) as wpool,
            tc.tile_pool(name="stage", bufs=3) as stpool,
            tc.tile_pool(name="zslab", bufs=2) as zpool,
            tc.tile_pool(name="uslab", bufs=4) as upool,
            tc.tile_pool(name="apsum", bufs=4, space="PSUM") as apsum,
            tc.tile_pool(name="trpsum", bufs=2, space="PSUM") as trpsum,
            tc.tile_pool(name="dpsum", bufs=2, space="PSUM") as dpsum,
        ):
            # ---- constants (idx planes first, in slices, so gathers
            # can start as soon as their columns land)
            idxA_t = cpool.tile([P, max(nAt, 1) * 8], i16)
            idxB_t = cpool.tile([P, max(nBt, 1) * 8], i16)
            slotp_t = cpool.tile([P, ntiles], f32)
            ISL = 64 * 8  # tiles' worth of idx columns per DMA slice
            for t_, d_ in ((idxA_t, idxA_d), (idxB_t, idxB_d)):
                ncols = t_.shape[1]
                for c0_ in range(0, ncols, ISL):
                    c1_ = min(c0_ + ISL, ncols)
                    nc.sync.dma_start(t_[:, c0_:c1_], d_[:, c0_:c1_])
            nc.sync.dma_start(slotp_t[:], slotp_d[:])
            iota_i = cpool.tile([P, P], i32)
            nc.gpsimd.iota(iota_i[:], pattern=[[1, P]], base=0,
                           channel_multiplier=0)
            iota_f = cpool.tile([P, P], f32)
            nc.vector.tensor_copy(iota_f[:], iota_i[:])
            ident = cpool.tile([P, P], f32)
            make_identity(nc, ident[:])
            ident_g = cpool.tile([P, P], gdt)
            nc.vector.tensor_copy(ident_g[:], ident[:])
            dinvw_t = cpool.tile([P, nb], f32)
            nc.sync.dma_start(dinvw_t[:], dinvw_d[:])
            w_t = []  # [stage][k][m] -> [128,128] f32r lhsT tiles
            for i in range(nw):
                tiles = []
                for k in range(kd):
                    row = []
                    for m in range(kd):
                        wt = cpool.tile([P, P], f32r, name=f"wt{i}_{k}_{m}",
                                        tag=f"wt{i}_{k}_{m}")
                        nc.sync.dma_start(
                            wt[:], w_d[i][k * P:(k + 1) * P, m * P:(m + 1) * P])
                        row.append(wt)
                    tiles.append(row)
                w_t.append(tiles)
            bias_t = []
            for i in range(nw):
                bt = cpool.tile([P, kd], f32, name=f"bt{i}", tag=f"bt{i}")
                nc.sync.dma_start(bt[:], bias_d[i][:])
                bias_t.append(bt)
            if has_bpre:
                c1rep_t = cpool.tile([P, npad], f32)
                nc.sync.dma_start(c1rep_t[:], c1rep_d[:])
                v1w_t = cpool.tile([P, kd], f32)
                nc.sync.dma_start(v1w_t[:], v1w_d[:])

            # ---- gather chunk machinery (flat per-plane, GCH tiles/call)
            qload = [0] * NSWQ  # greedy row-balance across SWDGE queues
            chunks = {"A": [], "B": []}
            issued = {"A": 0, "B": 0}
            plane_info = {
                "A": (tablo, idxA_t, nAt),
                "B": (tabhi, idxB_t, nBt),
            }

            def ensure_issued(plane, upto):
                tab_ap, idxp_t, ntot = plane_info[plane]
                upto = min(upto, ntot)
                while issued[plane] < upto:
                    c0 = issued[plane]
                    cn = min(GCH, ntot - c0)
                    gt = gpool.tile([P, GCH, d], gdt, tag="gch",
                                    name=f"g{plane}_{c0}")
                    q = qload.index(min(qload))
                    nc.gpsimd.dma_gather(
                        out_ap=gt[:, 0:cn, :], in_ap=tab_ap[:],
                        idxs_ap=idxp_t[:, c0 * 8:(c0 + cn) * 8],
                        num_idxs=cn * P, num_idxs_reg=cn * P, elem_size=d,
                        queue_num=q)
                    qload[q] += cn
                    chunks[plane].append(gt)
                    issued[plane] += cn

            def gtile(plane, t):
                return chunks[plane][t // GCH][:, t % GCH, :]

            # feature-major activation slabs, ring-buffered per dense slice
            uT_s = [None] * nsl

            dstate = {}

            def dense_stage1(sl):
                # pz matmuls + activation issue; PE-side ends here so the
                # Scalar relu overlaps the next block's scatter matmuls.
                s0 = sl * cfg.dense_n
                ns = min(cfg.dense_n, npad - s0)
                pz = [dpsum.tile([P, ns], f32, space="PSUM", tag="dps",
                                 name=f"pz{sl}_{dt}") for dt in range(kd)]
                for dt in range(kd):
                    for m in range(kd):
                        nc.tensor.matmul(
                            pz[dt][:], lhsT=w_t[0][m][dt][:],
                            rhs=uT_s[sl][:, m, 0:ns],
                            start=(m == 0), stop=(m == kd - 1))
                if has_bpre:
                    for dt in range(kd):
                        tmp = wpool.tile([P, cfg.dense_n], f32, tag="r1")
                        nc.vector.tensor_scalar_mul(
                            tmp[:, 0:ns], c1rep_t[:, s0:s0 + ns],
                            v1w_t[:, dt:dt + 1])
                        nc.vector.tensor_tensor(
                            out=pz[dt][:], in0=pz[dt][:], in1=tmp[:, 0:ns],
                            op=mybir.AluOpType.add)
                zdt = f32 if mode == 1 else f32r
                z = zpool.tile([P, kd, cfg.dense_n], zdt, tag="zr")
                for dt in range(kd):
                    nc.scalar.activation(
                        z[:, dt, 0:ns], pz[dt][:],
                        mybir.ActivationFunctionType.Relu,
                        bias=bias_t[0][:, dt:dt + 1], scale=1.0)
                dstate[sl] = z

            def dense_stage2(sl):
                s0 = sl * cfg.dense_n
                ns = min(cfg.dense_n, npad - s0)
                z = dstate.pop(sl)
                if mode == 1:
                    final = z
                else:
                    py = [dpsum.tile([P, ns], f32, space="PSUM", tag="dps",
                                     name=f"py{sl}_{dt}") for dt in range(kd)]
                    for dt in range(kd):
                        for m in range(kd):
                            nc.tensor.matmul(
                                py[dt][:], lhsT=w_t[1][m][dt][:],
                                rhs=z[:, m, 0:ns],
                                start=(m == 0), stop=(m == kd - 1))
                    final = zpool.tile([P, kd, cfg.dense_n], f32, tag="yT")
                    for dt in range(kd):
                        nc.scalar.activation(
                            final[:, dt, 0:ns], py[dt][:],
                            mybir.ActivationFunctionType.Identity,
                            bias=bias_t[1][:, dt:dt + 1], scale=1.0)

                for jj in range(ns // P):
                    blkj = (s0 + jj * P) // P
                    ost = stpool.tile([P, d], bf16, tag="ost")
                    for dt in range(kd):
                        ptr2 = trpsum.tile([P, P], f32, space="PSUM",
                                           tag="ptr")
                        nc.tensor.transpose(
                            out=ptr2[:], in_=final[:, dt, jj * P:(jj + 1) * P],
                            identity=ident[:])
                        nc.vector.tensor_copy(
                            ost[:, dt * P:(dt + 1) * P], ptr2[:])
                    nc.sync.dma_start(out_d[blkj * P:(blkj + 1) * P, :],
                                      ost[:])

            # ---- aggregation pass
            for b in range(nb):
                ta, tb = int(TA[b]), int(TB[b])
                tbt = ta + tb
                tcol = int(cumTA[b] + cumTB[b])
                bahead = min(b + LOOK, nb)
                ensure_issued("A", int(cumTA[bahead]))
                ensure_issued("B", int(cumTB[bahead]))

                psum_a = apsum.tile([P, d], f32, space="PSUM", tag="psum_a")
                if tbt:
                    p_all = ppool.tile([P, tmaxP, P], bf16, tag="pmat")
                    nc.vector.tensor_tensor(
                        out=p_all[:, 0:tbt, :],
                        in0=slotp_t[:, tcol:tcol + tbt, None].to_broadcast(
                            [P, tbt, P]),
                        in1=iota_f[:, None, :].to_broadcast([P, tbt, P]),
                        op=mybir.AluOpType.is_equal)
                selft = locpool.tile([P, d], gdt, tag="selft")
                nc.sync.dma_start(selft[:], loctab[b * P:(b + 1) * P, :])
                nc.tensor.matmul(psum_a[:], lhsT=ident_g[:], rhs=selft[:],
                                 start=True, stop=(tbt == 0))
                j = 0
                for t in range(ta):
                    nc.tensor.matmul(
                        psum_a[:], lhsT=p_all[:, j, :],
                        rhs=gtile("A", int(cumTA[b]) + t),
                        start=False, stop=(j == tbt - 1))
                    j += 1
                for t in range(tb):
                    nc.tensor.matmul(
                        psum_a[:], lhsT=p_all[:, j, :],
                        rhs=gtile("B", int(cumTB[b]) + t),
                        start=False, stop=(j == tbt - 1))
                    j += 1

                # epilogue: u' = psum * dinv (self-loops via loctab matmul)
                sl, off = divmod(b * P, cfg.dense_n)
                if off == 0:
                    uT_s[sl] = upool.tile(
                        [P, kd, min(cfg.dense_n, npad - sl * cfg.dense_n)],
                        f32r, tag="uslab", name=f"uTs{sl}")
                u2 = wpool.tile([P, d], f32, tag="u2")
                nc.scalar.mul(u2[:], psum_a[:], dinvw_t[:, b:b + 1])
                if DEBUG_U:
                    nc.sync.dma_start(dbg_d[b * P:(b + 1) * P, :], u2[:])
                for m in range(kd):
                    ptr = trpsum.tile([P, P], f32, space="PSUM", tag="ptr")
                    nc.tensor.transpose(out=ptr[:], in_=u2[:, m * P:(m + 1) * P],
                                        identity=ident[:])
                    nc.vector.tensor_copy(uT_s[sl][:, m, off:off + P], ptr[:])

                if DENSE_INTERLEAVE:
                    if b == nb - 1 or (b + 1) % blk_per_sl == 0:
                        dense_stage1(sl)
                    # run stage2 two blocks after its stage1 issued
                    sl2 = (b - 1) // blk_per_sl - 1
                    if (b - 1) % blk_per_sl == 0 and sl2 >= 0:
                        dense_stage2(sl2)
            if DENSE_INTERLEAVE:
                for sl2 in sorted(dstate):
                    dense_stage2(sl2)
            else:
                for sl2 in range(nsl):
                    dense_stage1(sl2)
                    dense_stage2(sl2)

    nc.compile()
    return nc


# ------------------------------------------------------------------ driver


def _run(cfg, nc_prog, per_core_common, per_core_vars, trace=False):
    in_maps = []
    for c in range(cfg.nc):
        m = dict(per_core_common)
        m.update(per_core_vars[c])
        in_maps.append(m)
    res = run_bass_kernel_spmd(nc_prog, in_maps, core_ids=list(range(cfg.nc)),
                               trace=trace)
    return res


def gcn_forward(cfg, x, edge_index, W_pre, b_pre, W1, b1, W2, b2, W_post,
                b_post, trace=False, ret_times=None):
    x = np.asarray(x, np.float32)
    src = np.asarray(edge_index[0], np.int64)
    dst = np.asarray(edge_index[1], np.int64)
    W_pre, W1, W2, W_post = (np.asarray(w, np.float32)
                             for w in (W_pre, W1, W2, W_post))
    b_pre, b1, b2, b_post = (np.asarray(b, np.float32)
                             for b in (b_pre, b1, b2, b_post))

    n, d, nl, nb, npad = cfg.n_nodes, cfg.d, cfg.nloc, cfg.nblk, cfg.npad
    deg = (np.bincount(dst, minlength=n) + 1).astype(np.float64)
    dinv = (1.0 / np.sqrt(deg)).astype(np.float32)

    TA, TB, edge_planes = _prep_edges(cfg, src, dst)

    def local_pad(tab, c):
        out = np.zeros((npad, d), tab.dtype)
        out[:nl] = tab[c * nl:(c + 1) * nl]
        return out

    xs = x * dinv[:, None]
    WA = (W_pre.astype(np.float64) @ W1.astype(np.float64)).astype(np.float32)

    has_bpre = bool(np.any(b_pre != 0))
    dinv_cols = [
        _wrap_cols(dinv[c * nl:(c + 1) * nl], nb, npad) for c in range(cfg.nc)]

    # ---------- launch 1
    prog1 = build_launch(cfg, 1, TA, TB, has_bpre=has_bpre)
    tdt = ml_dtypes.bfloat16
    common1 = {
        "tablo": xs[: cfg.split].astype(tdt),
        "tabhi": xs[cfg.n_nodes - cfg.nhi:].astype(tdt),
        "w0": WA,
        "bias0": b1.reshape(d // P, P).T.copy(),
    }
    if has_bpre:
        v1 = (b_pre.astype(np.float64) @ W1.astype(np.float64)).astype(
            np.float32)
        common1["v1w"] = v1.reshape(d // P, P).T.copy()
        # c1[dst] = (s[dst] + dinv[dst]) * dinv[dst],  s = sum_e dinv[src]
        s = np.zeros(n, np.float64)
        np.add.at(s, dst, dinv[src].astype(np.float64))
        c1_full = ((s + dinv) * dinv).astype(np.float32)
    vars1 = []
    for c in range(cfg.nc):
        v = {
            "loctab": local_pad(xs.astype(tdt), c),
            "idxA": edge_planes[c]["idxA"],
            "idxB": edge_planes[c]["idxB"],
            "slotp": edge_planes[c]["slotp"],
            "dinvw": dinv_cols[c],
        }
        if has_bpre:
            cl = np.zeros(npad, np.float32)
            cl[:nl] = c1_full[c * nl:(c + 1) * nl]
            v["c1rep"] = np.tile(cl, (P, 1))
        vars1.append(v)
    res1 = _run(cfg, prog1, common1, vars1, trace=trace)
    g1 = np.concatenate(
        [res1.results[c]["out"][:nl].astype(np.float32)
         for c in range(cfg.nc)])
    g1 *= dinv[:, None]
    if ret_times is not None:
        ret_times.append(res1.exec_time_ns)

    # ---------- launch 2
    prog2 = build_launch(cfg, 2, TA, TB, has_bpre=False)
    common2 = {
        "tablo": g1[: cfg.split].astype(tdt),
        "tabhi": g1[cfg.n_nodes - cfg.nhi:].astype(tdt),
        "w0": W2,
        "w1": W_post,
        "bias0": b2.reshape(d // P, P).T.copy(),
        "bias1": b_post.reshape(d // P, P).T.copy(),
    }
    vars2 = []
    for c in range(cfg.nc):
        vars2.append({
            "loctab": local_pad(g1.astype(tdt), c),
            "idxA": edge_planes[c]["idxA"],
            "idxB": edge_planes[c]["idxB"],
            "slotp": edge_planes[c]["slotp"],
            "dinvw": dinv_cols[c],
        })
    res2 = _run(cfg, prog2, common2, vars2, trace=trace)
    y = np.concatenate(
        [res2.results[c]["out"][:nl].astype(np.float32)
         for c in range(cfg.nc)])
    if ret_times is not None:
        ret_times.append(res2.exec_time_ns)
    return y


def kernel(x, edge_index, W_pre, b_pre, W1, b1, W2, b2, W_post, b_post):
    cfg = Cfg()
    return gcn_forward(cfg, x, edge_index, W_pre, b_pre, W1, b1, W2, b2,
                       W_post, b_post)
